# revision 7
# baseline (speedup 1.0000x reference)
"""GCN graph-classification kernel for 8 Trainium2 NeuronCores.

Strategy:
  - Nodes sharded contiguously across 8 cores (8192/core, 64 dst-blocks of 128).
  - Edges partitioned by dst block, split by src half (table is split in two
    32768-row halves so dma_gather's int16 indices reach every row), sorted,
    packed into 128-edge tiles with a host-built weighted one-hot "selector"
    per tile (degree norms baked in), so segment-sum == selector^T @
    gathered_rows on the TensorEngine.
  - Per layer: two chunked AllGathers (lo/hi halves of h, bf16) into
    replicated HBM tables, dma_gather row gathers, selector matmuls -> agg
    (PSUM), PE-transpose agg, dense matmul with conv_w, BatchNorm via
    ones-matmul stats + 4KB AllReduce, fused scale/bias + ReLU. The hi-half
    AllGather overlaps with lo-half gather/compute.
  - Mean-pool per graph via host-built pool-selector matmuls + AllReduce,
    then a replicated MLP head; core 0's output is returned.
"""

import sys

for _p in ("/opt/trn_rl_repo",):
    if _p not in sys.path:
        sys.path.insert(0, _p)

import numpy as np
import ml_dtypes

BF16 = ml_dtypes.bfloat16

N_NODES, N_EDGES, IN_FEATS, HID = 65536, 524288, 128, 512
N_CLASSES, N_GRAPHS, N_LAYERS = 64, 256, 4
BN_EPS = 1e-5
NEG_INF = -1.0e12

N_CORES = 8
NPC = N_NODES // N_CORES          # 8192 nodes per core
NBLK = NPC // 128                 # 64 dst blocks per core
HNPC = NPC // 2                   # 4096: nodes per core per half
P = 128

_CACHE = {}


# --------------------------------------------------------------------------
# Host-side preprocessing (index/layout manipulation only)
# --------------------------------------------------------------------------

def _table_row(n):
    """Row of global node n in the split gather table (per half)."""
    c = n // NPC
    i = n % NPC
    return c * HNPC + (i % HNPC)


def _host_prep(x, src, dst, graph_ids, observed,
               enc_b, bn_gamma, bn_beta, mlp_b0, mlp_b1, cls_b):
    src = np.asarray(src)
    dst = np.asarray(dst)
    graph_ids = np.asarray(graph_ids)

    out_deg = np.bincount(src, minlength=N_NODES).clip(1).astype(np.float32)
    in_deg = np.bincount(dst, minlength=N_NODES).clip(1).astype(np.float32)
    w_edge = ((out_deg[src] ** -0.5) * (in_deg[dst] ** -0.5)).astype(np.float32)

    half = (src % NPC) // HNPC                       # 0 = lo table, 1 = hi
    # sort edges by (dst block, half, src) -> contiguous (block, half) runs
    b_g_all = dst // P
    order = np.lexsort((src, half, b_g_all))
    dst_s = dst[order]
    src_s = src[order]
    w_s = w_edge[order]
    half_s = half[order]
    b_g = dst_s // P

    n_gblk = N_NODES // P
    # counts per (block, half)
    key = b_g * 2 + half_s
    cnt = np.bincount(key, minlength=n_gblk * 2).reshape(n_gblk, 2)
    T_LO = int(np.ceil(cnt[:, 0].max() / P))
    T_HI = int(np.ceil(cnt[:, 1].max() / P))
    n_lo, n_hi = T_LO * P, T_HI * P
    T_TOT = T_LO + T_HI

    # slot within (block, half) group
    grp_starts = np.zeros(n_gblk * 2 + 1, np.int64)
    np.cumsum(cnt.reshape(-1), out=grp_starts[1:])
    s = np.arange(N_EDGES) - grp_starts[key]
    t = s // P
    p = s % P
    core = b_g // NBLK
    b = b_g % NBLK

    row = _table_row(src_s).astype(np.int64)

    # gather index arrays, int16, 16-partition wrap replicated to 128
    C_LO, C_HI = n_lo // 16, n_hi // 16
    idx_lo16 = np.zeros((N_CORES, 16, NBLK * C_LO), np.int16)
    idx_hi16 = np.zeros((N_CORES, 16, NBLK * C_HI), np.int16)
    lo = half_s == 0
    hi = ~lo
    idx_lo16[core[lo], s[lo] % 16, b[lo] * C_LO + s[lo] // 16] = row[lo]
    idx_hi16[core[hi], s[hi] % 16, b[hi] * C_HI + s[hi] // 16] = row[hi]
    idx_lo = np.tile(idx_lo16, (1, 8, 1))
    idx_hi = np.tile(idx_hi16, (1, 8, 1))

    # selectors: per block, T_LO lo tiles then T_HI hi tiles
    tt = np.where(lo, t, T_LO + t)
    col = b * T_TOT + tt
    sel_host = np.zeros((N_CORES, P, NBLK * T_TOT * P), BF16)
    sel_host[core, p, col * P + (dst_s - b_g * P)] = w_s.astype(BF16)

    # pool selector
    g_cnt = np.bincount(graph_ids, minlength=N_GRAPHS).clip(1).astype(np.float32)
    inv_cnt = (1.0 / g_cnt).astype(np.float32)
    nodes = np.arange(N_NODES)
    nc_ = nodes // NPC
    r = nodes % NPC
    psel_host = np.zeros((N_CORES, P, NBLK * N_GRAPHS), BF16)
    psel_host[nc_, r % P, (r // P) * N_GRAPHS + graph_ids] = \
        inv_cnt[graph_ids].astype(BF16)

    xT = np.ascontiguousarray(np.asarray(x).T)           # [128, N]

    obs = np.asarray(observed).astype(np.float32)
    mask_mul = obs.reshape(N_CLASSES, 1)
    mask_add = ((1.0 - obs) * NEG_INF).astype(np.float32).reshape(N_CLASSES, 1)

    shaped = dict(
        enc_b=np.asarray(enc_b, np.float32).reshape(1, HID),
        bn_gamma=np.asarray(bn_gamma, np.float32).reshape(1, N_LAYERS * HID),
        bn_beta=np.asarray(bn_beta, np.float32).reshape(1, N_LAYERS * HID),
        mlp_b0=np.ascontiguousarray(
            np.asarray(mlp_b0, np.float32).reshape(2, P).T),      # [128, 2]
        mlp_b1=np.asarray(mlp_b1, np.float32).reshape(P, 1),
        cls_b=np.asarray(cls_b, np.float32).reshape(N_CLASSES, 1),
        mask_mul=mask_mul, mask_add=mask_add,
    )
    return dict(T_LO=T_LO, T_HI=T_HI, idx_lo=idx_lo, idx_hi=idx_hi,
                sel=sel_host, psel=psel_host, xT=xT, shaped=shaped)


# --------------------------------------------------------------------------
# Device program
# --------------------------------------------------------------------------

def _build_program(T_LO, T_HI):
    import concourse.bass as bass
    import concourse.bacc as bacc
    import concourse.tile as tile
    import concourse.mybir as mybir
    from concourse.masks import make_identity

    dt = mybir.dt
    AF = mybir.ActivationFunctionType
    OP = mybir.AluOpType

    T_TOT = T_LO + T_HI
    n_lo, n_hi = T_LO * P, T_HI * P
    C_LO, C_HI = n_lo // 16, n_hi // 16
    NTAB = N_NODES // 2

    nc = bacc.Bacc("TRN2", target_bir_lowering=False, debug=False,
                   num_devices=N_CORES)

    # ---- DRAM I/O ----
    xT_in = nc.dram_tensor("xT", [P, NPC], dt.float32, kind="ExternalInput")
    idxlo_in = nc.dram_tensor("idx_lo", [P, NBLK * C_LO], dt.int16,
                              kind="ExternalInput")
    idxhi_in = nc.dram_tensor("idx_hi", [P, NBLK * C_HI], dt.int16,
                              kind="ExternalInput")
    sel_in = nc.dram_tensor("sel", [P, NBLK * T_TOT * P], dt.bfloat16,
                            kind="ExternalInput")
    psel_in = nc.dram_tensor("psel", [P, NBLK * N_GRAPHS], dt.bfloat16,
                             kind="ExternalInput")
    encw_in = nc.dram_tensor("enc_w", [IN_FEATS, HID], dt.float32,
                             kind="ExternalInput")
    encb_in = nc.dram_tensor("enc_b", [1, HID], dt.float32, kind="ExternalInput")
    convw_in = nc.dram_tensor("conv_w", [N_LAYERS, HID, HID], dt.float32,
                              kind="ExternalInput")
    gam_in = nc.dram_tensor("bn_gamma", [1, N_LAYERS * HID], dt.float32,
                            kind="ExternalInput")
    bet_in = nc.dram_tensor("bn_beta", [1, N_LAYERS * HID], dt.float32,
                            kind="ExternalInput")
    w0_in = nc.dram_tensor("mlp_w0", [HID, 256], dt.float32, kind="ExternalInput")
    b0_in = nc.dram_tensor("mlp_b0", [P, 2], dt.float32, kind="ExternalInput")
    w1_in = nc.dram_tensor("mlp_w1", [256, P], dt.float32, kind="ExternalInput")
    b1_in = nc.dram_tensor("mlp_b1", [P, 1], dt.float32, kind="ExternalInput")
    clsw_in = nc.dram_tensor("cls_w", [P, N_CLASSES], dt.float32,
                             kind="ExternalInput")
    clsb_in = nc.dram_tensor("cls_b", [N_CLASSES, 1], dt.float32,
                             kind="ExternalInput")
    mmul_in = nc.dram_tensor("mask_mul", [N_CLASSES, 1], dt.float32,
                             kind="ExternalInput")
    madd_in = nc.dram_tensor("mask_add", [N_CLASSES, 1], dt.float32,
                             kind="ExternalInput")

    hd_lo = nc.dram_tensor("hd_lo", [HNPC, HID], dt.bfloat16, kind="Internal")
    hd_hi = nc.dram_tensor("hd_hi", [HNPC, HID], dt.bfloat16, kind="Internal")
    hs_lo = nc.dram_tensor("hs_lo", [NTAB, HID], dt.bfloat16,
                           kind="Internal", addr_space="Shared")
    hs_hi = nc.dram_tensor("hs_hi", [NTAB, HID], dt.bfloat16,
                           kind="Internal", addr_space="Shared")
    stats_loc = nc.dram_tensor("stats_loc", [1, 2 * HID], dt.float32,
                               kind="Internal")
    stats_glob = nc.dram_tensor("stats_glob", [1, 2 * HID], dt.float32,
                                kind="Internal", addr_space="Shared")
    pool_loc = nc.dram_tensor("pool_loc", [HID, N_GRAPHS], dt.float32,
                              kind="Internal")
    pool_glob = nc.dram_tensor("pool_glob", [HID, N_GRAPHS], dt.float32,
                               kind="Internal", addr_space="Shared")
    out_ext = nc.dram_tensor("logits_out", [N_CLASSES, N_GRAPHS], dt.float32,
                             kind="ExternalOutput")

    RG = [list(range(N_CORES))]
    HB = NBLK // 2                                   # 32 blocks per half

    with tile.TileContext(nc) as tc:
        with tc.tile_pool(name="const", bufs=1) as cpool, \
             tc.tile_pool(name="hbuf", bufs=1) as hpool, \
             tc.tile_pool(name="small", bufs=1) as spool:

            # ---- resident constants ----
            xT_sb = cpool.tile([P, NPC], dt.bfloat16)
            nc.gpsimd.dma_start(out=xT_sb[:], in_=xT_in[:])
            idxlo_sb = cpool.tile([P, NBLK * C_LO], dt.int16)
            nc.sync.dma_start(out=idxlo_sb[:], in_=idxlo_in[:])
            idxhi_sb = cpool.tile([P, NBLK * C_HI], dt.int16)
            nc.sync.dma_start(out=idxhi_sb[:], in_=idxhi_in[:])
            encw_sb = cpool.tile([IN_FEATS, HID], dt.bfloat16)
            nc.gpsimd.dma_start(out=encw_sb[:], in_=encw_in[:])
            convw_sb = cpool.tile([P, N_LAYERS * 4 * HID], dt.bfloat16)
            for l in range(N_LAYERS):
                for kb in range(4):
                    nc.gpsimd.dma_start(
                        out=convw_sb[:, (l * 4 + kb) * HID:(l * 4 + kb + 1) * HID],
                        in_=convw_in[l, kb * P:(kb + 1) * P, :])
            gam_sb = cpool.tile([1, N_LAYERS * HID], dt.float32)
            nc.sync.dma_start(out=gam_sb[:], in_=gam_in[:])
            bet_sb = cpool.tile([1, N_LAYERS * HID], dt.float32)
            nc.sync.dma_start(out=bet_sb[:], in_=bet_in[:])
            ident_sb = cpool.tile([P, P], dt.bfloat16)
            make_identity(nc, ident_sb[:])
            ones_col = cpool.tile([P, 1], dt.bfloat16)
            nc.vector.memset(ones_col[:], 1.0)
            ones_row = cpool.tile([1, P], dt.bfloat16)
            nc.vector.memset(ones_row[:], 1.0)
            eps_t = cpool.tile([1, 1], dt.float32)
            nc.vector.memset(eps_t[:], BN_EPS)

            # head weights
            w0_sb = cpool.tile([P, 4 * 256], dt.bfloat16)
            nc.gpsimd.dma_start(
                out=w0_sb[:].rearrange("p (kb m) -> p kb m", m=256),
                in_=w0_in[:].rearrange("(kb p) m -> p kb m", p=P))
            b0_sb = cpool.tile([P, 2], dt.float32)
            nc.sync.dma_start(out=b0_sb[:], in_=b0_in[:])
            w1_sb = cpool.tile([P, 2 * P], dt.bfloat16)
            nc.gpsimd.dma_start(
                out=w1_sb[:].rearrange("p (kb m) -> p kb m", m=P),
                in_=w1_in[:].rearrange("(kb p) m -> p kb m", p=P))
            b1_sb = cpool.tile([P, 1], dt.float32)
            nc.sync.dma_start(out=b1_sb[:], in_=b1_in[:])
            clsw_sb = cpool.tile([P, N_CLASSES], dt.bfloat16)
            nc.gpsimd.dma_start(out=clsw_sb[:], in_=clsw_in[:])
            clsb_sb = cpool.tile([N_CLASSES, 1], dt.float32)
            nc.sync.dma_start(out=clsb_sb[:], in_=clsb_in[:])
            mmul_sb = cpool.tile([N_CLASSES, 1], dt.float32)
            nc.sync.dma_start(out=mmul_sb[:], in_=mmul_in[:])
            madd_sb = cpool.tile([N_CLASSES, 1], dt.float32)
            nc.sync.dma_start(out=madd_sb[:], in_=madd_in[:])

            h_sb = hpool.tile([P, NBLK * HID], dt.bfloat16)

            encb_row = cpool.tile([1, HID], dt.bfloat16)
            nc.gpsimd.dma_start(out=encb_row[:], in_=encb_in[:])
            encb_bc = cpool.tile([P, HID], dt.bfloat16)

            def emit_h_dma_ag(which):
                """DMA one half of h to HBM and AllGather it."""
                if which == 0:
                    nc.sync.dma_start(
                        out=hd_lo[:].rearrange("(b p) f -> p b f", p=P),
                        in_=h_sb[:, :HB * HID].rearrange(
                            "p (b f) -> p b f", f=HID))
                    nc.gpsimd.collective_compute(
                        "AllGather", OP.bypass, replica_groups=RG,
                        ins=[hd_lo[:]], outs=[hs_lo[:]])
                else:
                    nc.sync.dma_start(
                        out=hd_hi[:].rearrange("(b p) f -> p b f", p=P),
                        in_=h_sb[:, HB * HID:].rearrange(
                            "p (b f) -> p b f", f=HID))
                    nc.gpsimd.collective_compute(
                        "AllGather", OP.bypass, replica_groups=RG,
                        ins=[hd_hi[:]], outs=[hs_hi[:]])

            # ---- encoder ----
            with tc.tile_pool(name="psenc", bufs=2, space="PSUM") as pse:
                pbc = pse.tile([P, HID], dt.float32, tag="enc")
                nc.tensor.matmul(out=pbc[:], lhsT=ones_row[:], rhs=encb_row[:],
                                 start=True, stop=True)
                nc.vector.tensor_copy(out=encb_bc[:], in_=pbc[:])
                for b in range(NBLK):
                    pe = pse.tile([P, HID], dt.float32, tag="enc")
                    nc.tensor.matmul(out=pe[:],
                                     lhsT=xT_sb[:, b * P:(b + 1) * P],
                                     rhs=encw_sb[:], start=True, stop=True)
                    nc.vector.tensor_tensor(
                        out=h_sb[:, b * HID:(b + 1) * HID],
                        in0=pe[:], in1=encb_bc[:], op=OP.add)
                    if b == HB - 1:
                        emit_h_dma_ag(0)
                emit_h_dma_ag(1)

            # ---- GCN layers ----
            with tc.tile_pool(name="selp", bufs=2) as selp, \
                 tc.tile_pool(name="gathp", bufs=2) as gathp, \
                 tc.tile_pool(name="aggp", bufs=2) as aggp, \
                 tc.tile_pool(name="ps_agg", bufs=2, space="PSUM") as ps_agg, \
                 tc.tile_pool(name="ps_tr", bufs=2, space="PSUM") as ps_tr, \
                 tc.tile_pool(name="ps_rst", bufs=2, space="PSUM") as ps_rst, \
                 tc.tile_pool(name="ps_st", bufs=1, space="PSUM") as ps_st:

                for l in range(N_LAYERS):
                    psum_sum = ps_st.tile([1, HID], dt.float32, tag="s0")
                    psum_sq = ps_st.tile([1, HID], dt.float32, tag="s1")

                    for b in range(NBLK):
                        sel_t = selp.tile([P, T_TOT * P], dt.bfloat16, tag="sel")
                        nc.sync.dma_start(
                            out=sel_t[:],
                            in_=sel_in[:, b * T_TOT * P:(b + 1) * T_TOT * P])
                        g_lo = gathp.tile([P, T_LO * HID], dt.bfloat16,
                                          tag="glo")
                        nc.gpsimd.dma_gather(
                            out_ap=g_lo[:].rearrange("p (t e) -> p t e", e=HID),
                            in_ap=hs_lo[:],
                            idxs_ap=idxlo_sb[:, b * C_LO:(b + 1) * C_LO],
                            num_idxs=n_lo, num_idxs_reg=n_lo, elem_size=HID)
                        g_hi = gathp.tile([P, T_HI * HID], dt.bfloat16,
                                          tag="ghi")
                        nc.gpsimd.dma_gather(
                            out_ap=g_hi[:].rearrange("p (t e) -> p t e", e=HID),
                            in_ap=hs_hi[:],
                            idxs_ap=idxhi_sb[:, b * C_HI:(b + 1) * C_HI],
                            num_idxs=n_hi, num_idxs_reg=n_hi, elem_size=HID)
                        pagg = ps_agg.tile([P, HID], dt.float32, tag="agg")
                        for t in range(T_TOT):
                            rhs = (g_lo[:, t * HID:(t + 1) * HID] if t < T_LO
                                   else g_hi[:, (t - T_LO) * HID:
                                             (t - T_LO + 1) * HID])
                            nc.tensor.matmul(
                                out=pagg[:],
                                lhsT=sel_t[:, t * P:(t + 1) * P],
                                rhs=rhs, start=(t == 0), stop=(t == T_TOT - 1))
                        agg_sb = aggp.tile([P, HID], dt.bfloat16, tag="agg_sb")
                        nc.vector.tensor_copy(out=agg_sb[:], in_=pagg[:])
                        ptr = ps_tr.tile([P, HID], dt.bfloat16, tag="tr")
                        for fb in range(4):
                            nc.tensor.transpose(
                                out=ptr[:, fb * P:(fb + 1) * P],
                                in_=agg_sb[:, fb * P:(fb + 1) * P],
                                identity=ident_sb[:])
                        aggT_sb = aggp.tile([P, HID], dt.bfloat16, tag="aggT")
                        nc.vector.tensor_copy(out=aggT_sb[:], in_=ptr[:])
                        prst = ps_rst.tile([P, HID], dt.float32, tag="rst")
                        for kb in range(4):
                            nc.tensor.matmul(
                                out=prst[:],
                                lhsT=aggT_sb[:, kb * P:(kb + 1) * P],
                                rhs=convw_sb[:, (l * 4 + kb) * HID:
                                             (l * 4 + kb + 1) * HID],
                                start=(kb == 0), stop=(kb == 3))
                        nc.vector.tensor_copy(
                            out=h_sb[:, b * HID:(b + 1) * HID], in_=prst[:])
                        nc.tensor.matmul(out=psum_sum[:], lhsT=ones_col[:],
                                         rhs=h_sb[:, b * HID:(b + 1) * HID],
                                         start=(b == 0), stop=(b == NBLK - 1))
                        sq_t = aggp.tile([P, HID], dt.bfloat16, tag="sq")
                        nc.scalar.square(out=sq_t[:],
                                         in_=h_sb[:, b * HID:(b + 1) * HID])
                        nc.tensor.matmul(out=psum_sq[:], lhsT=ones_col[:],
                                         rhs=sq_t[:],
                                         start=(b == 0), stop=(b == NBLK - 1))

                    # stats -> AllReduce -> scale/bias
                    stats_sb = spool.tile([1, 2 * HID], dt.float32, tag="st")
                    nc.vector.tensor_copy(out=stats_sb[:, :HID], in_=psum_sum[:])
                    nc.vector.tensor_copy(out=stats_sb[:, HID:], in_=psum_sq[:])
                    nc.sync.dma_start(out=stats_loc[:], in_=stats_sb[:])
                    nc.gpsimd.collective_compute(
                        "AllReduce", OP.add, replica_groups=RG,
                        ins=[stats_loc[:]], outs=[stats_glob[:]])
                    stg = spool.tile([1, 2 * HID], dt.float32, tag="stg")
                    nc.sync.dma_start(out=stg[:], in_=stats_glob[:])
                    mean_r = spool.tile([1, HID], dt.float32, tag="r0")
                    nc.vector.tensor_scalar_mul(mean_r[:], stg[:, :HID],
                                                1.0 / N_NODES)
                    ex2_r = spool.tile([1, HID], dt.float32, tag="r1")
                    nc.vector.tensor_scalar_mul(ex2_r[:], stg[:, HID:],
                                                1.0 / N_NODES)
                    var_r = spool.tile([1, HID], dt.float32, tag="r2")
                    nc.vector.tensor_tensor(out=var_r[:], in0=mean_r[:],
                                            in1=mean_r[:], op=OP.mult)
                    nc.vector.tensor_tensor(out=var_r[:], in0=ex2_r[:],
                                            in1=var_r[:], op=OP.subtract)
                    std_r = spool.tile([1, HID], dt.float32, tag="r3")
                    nc.scalar.activation(out=std_r[:], in_=var_r[:],
                                         func=AF.Sqrt, bias=eps_t[:])
                    rstd_r = spool.tile([1, HID], dt.float32, tag="r4")
                    nc.vector.reciprocal(out=rstd_r[:], in_=std_r[:])
                    scale_r = spool.tile([1, HID], dt.float32, tag="r5")
                    nc.vector.tensor_tensor(
                        out=scale_r[:], in0=rstd_r[:],
                        in1=gam_sb[:, l * HID:(l + 1) * HID], op=OP.mult)
                    bias2_r = spool.tile([1, HID], dt.float32, tag="r6")
                    nc.vector.tensor_tensor(out=bias2_r[:], in0=mean_r[:],
                                            in1=scale_r[:], op=OP.mult)
                    nc.vector.tensor_tensor(
                        out=bias2_r[:], in0=bet_sb[:, l * HID:(l + 1) * HID],
                        in1=bias2_r[:], op=OP.subtract)
                    scale_bf = spool.tile([1, HID], dt.bfloat16, tag="r7")
                    nc.vector.tensor_copy(out=scale_bf[:], in_=scale_r[:])
                    bias2_bf = spool.tile([1, HID], dt.bfloat16, tag="r8")
                    nc.vector.tensor_copy(out=bias2_bf[:], in_=bias2_r[:])
                    scale_bc = spool.tile([P, HID], dt.bfloat16, tag="bc0")
                    bias2_bc = spool.tile([P, HID], dt.bfloat16, tag="bc1")
                    ptr0 = ps_tr.tile([P, HID], dt.float32, tag="tr")
                    nc.tensor.matmul(out=ptr0[:], lhsT=ones_row[:],
                                     rhs=scale_bf[:], start=True, stop=True)
                    nc.vector.tensor_copy(out=scale_bc[:], in_=ptr0[:])
                    ptr1 = ps_tr.tile([P, HID], dt.float32, tag="tr")
                    nc.tensor.matmul(out=ptr1[:], lhsT=ones_row[:],
                                     rhs=bias2_bf[:], start=True, stop=True)
                    nc.vector.tensor_copy(out=bias2_bc[:], in_=ptr1[:])

                    # BN apply + relu in place; kick off next layer's
                    # AllGather per half as soon as that half is done
                    for b in range(NBLK):
                        tmp_t = aggp.tile([P, HID], dt.bfloat16, tag="bn")
                        nc.vector.tensor_tensor(
                            out=tmp_t[:], in0=h_sb[:, b * HID:(b + 1) * HID],
                            in1=scale_bc[:], op=OP.mult)
                        nc.vector.tensor_tensor(out=tmp_t[:], in0=tmp_t[:],
                                                in1=bias2_bc[:], op=OP.add)
                        nc.scalar.activation(
                            out=h_sb[:, b * HID:(b + 1) * HID], in_=tmp_t[:],
                            func=AF.Relu)
                        if l < N_LAYERS - 1 and b == HB - 1:
                            emit_h_dma_ag(0)
                    if l < N_LAYERS - 1:
                        emit_h_dma_ag(1)

            # ---- graph mean pool ----
            pool_sb = spool.tile([P, 4 * N_GRAPHS], dt.float32, tag="pool")
            with tc.tile_pool(name="ps_pool", bufs=1, space="PSUM") as psp, \
                 tc.tile_pool(name="pselp", bufs=2) as pselp:
                ppool = [psp.tile([P, N_GRAPHS], dt.float32, tag=f"pp{fb}",
                                  name=f"ppool{fb}")
                         for fb in range(4)]
                for b in range(NBLK):
                    psel_t = pselp.tile([P, N_GRAPHS], dt.bfloat16, tag="ps")
                    nc.sync.dma_start(
                        out=psel_t[:],
                        in_=psel_in[:, b * N_GRAPHS:(b + 1) * N_GRAPHS])
                    for fb in range(4):
                        nc.tensor.matmul(
                            out=ppool[fb][:],
                            lhsT=h_sb[:, b * HID + fb * P:b * HID + (fb + 1) * P],
                            rhs=psel_t[:],
                            start=(b == 0), stop=(b == NBLK - 1))
                for fb in range(4):
                    nc.vector.tensor_copy(
                        out=pool_sb[:, fb * N_GRAPHS:(fb + 1) * N_GRAPHS],
                        in_=ppool[fb][:])
            nc.sync.dma_start(
                out=pool_loc[:].rearrange("(fb p) g -> p fb g", p=P),
                in_=pool_sb[:].rearrange("p (fb g) -> p fb g", g=N_GRAPHS))
            nc.gpsimd.collective_compute(
                "AllReduce", OP.add, replica_groups=RG,
                ins=[pool_loc[:]], outs=[pool_glob[:]])
            poolg_sb = spool.tile([P, 4 * N_GRAPHS], dt.bfloat16, tag="poolg")
            nc.gpsimd.dma_start(
                out=poolg_sb[:].rearrange("p (fb g) -> p fb g", g=N_GRAPHS),
                in_=pool_glob[:].rearrange("(fb p) g -> p fb g", p=P))

            # ---- MLP head ----
            with tc.tile_pool(name="ps_head", bufs=1, space="PSUM") as psh:
                hg1_sb = spool.tile([P, 2 * N_GRAPHS], dt.bfloat16, tag="hg1")
                for mb in range(2):
                    ph1 = psh.tile([P, N_GRAPHS], dt.float32, tag="h1")
                    for kb in range(4):
                        nc.tensor.matmul(
                            out=ph1[:],
                            lhsT=w0_sb[:, kb * 256 + mb * P:kb * 256 + (mb + 1) * P],
                            rhs=poolg_sb[:, kb * N_GRAPHS:(kb + 1) * N_GRAPHS],
                            start=(kb == 0), stop=(kb == 3))
                    nc.scalar.activation(
                        out=hg1_sb[:, mb * N_GRAPHS:(mb + 1) * N_GRAPHS],
                        in_=ph1[:], func=AF.Relu, bias=b0_sb[:, mb:mb + 1])
                ph2 = psh.tile([P, N_GRAPHS], dt.float32, tag="h2")
                for kb in range(2):
                    nc.tensor.matmul(
                        out=ph2[:], lhsT=w1_sb[:, kb * P:(kb + 1) * P],
                        rhs=hg1_sb[:, kb * N_GRAPHS:(kb + 1) * N_GRAPHS],
                        start=(kb == 0), stop=(kb == 1))
                hg2_sb = spool.tile([P, N_GRAPHS], dt.bfloat16, tag="hg2")
                nc.scalar.activation(out=hg2_sb[:], in_=ph2[:], func=AF.Relu,
                                     bias=b1_sb[:])
                plg = psh.tile([N_CLASSES, N_GRAPHS], dt.float32, tag="lg")
                nc.tensor.matmul(out=plg[:], lhsT=clsw_sb[:], rhs=hg2_sb[:],
                                 start=True, stop=True)
                lg_sb = spool.tile([N_CLASSES, N_GRAPHS], dt.float32, tag="lgs")
                nc.vector.tensor_scalar(out=lg_sb[:], in0=plg[:],
                                        scalar1=clsb_sb[:], scalar2=None,
                                        op0=mybir.AluOpType.add)
                nc.vector.tensor_scalar(out=lg_sb[:], in0=lg_sb[:],
                                        scalar1=mmul_sb[:], scalar2=madd_sb[:],
                                        op0=mybir.AluOpType.mult,
                                        op1=mybir.AluOpType.add)
                nc.sync.dma_start(out=out_ext[:], in_=lg_sb[:])

    nc.compile()
    return nc


def _get_program(T_LO, T_HI):
    key = (T_LO, T_HI)
    if key not in _CACHE:
        _CACHE[key] = _build_program(T_LO, T_HI)
    return _CACHE[key]


def _make_in_maps(prep, inputs):
    xT = prep["xT"].astype(np.float32)
    in_maps = []
    for c in range(N_CORES):
        in_maps.append(dict(
            xT=np.ascontiguousarray(xT[:, c * NPC:(c + 1) * NPC]),
            idx_lo=prep["idx_lo"][c], idx_hi=prep["idx_hi"][c],
            sel=prep["sel"][c], psel=prep["psel"][c],
            enc_w=np.asarray(inputs["enc_w"], np.float32),
            conv_w=np.asarray(inputs["conv_w"], np.float32),
            mlp_w0=np.asarray(inputs["mlp_w0"], np.float32),
            mlp_w1=np.asarray(inputs["mlp_w1"], np.float32),
            cls_w=np.asarray(inputs["cls_w"], np.float32),
            **prep["shaped"],
        ))
    return in_maps


# --------------------------------------------------------------------------
# Entry point
# --------------------------------------------------------------------------

def kernel(x, src, dst, graph_ids, num_graphs, enc_w, enc_b, conv_w,
           bn_gamma, bn_beta, mlp_w0, mlp_b0, mlp_w1, mlp_b1,
           cls_w, cls_b, observed):
    from concourse import bass_utils

    assert int(num_graphs) == N_GRAPHS
    prep = _host_prep(x, src, dst, graph_ids, observed,
                      enc_b, bn_gamma, bn_beta, mlp_b0, mlp_b1, cls_b)
    nc = _get_program(prep["T_LO"], prep["T_HI"])
    in_maps = _make_in_maps(prep, dict(
        enc_w=enc_w, conv_w=conv_w, mlp_w0=mlp_w0, mlp_w1=mlp_w1, cls_w=cls_w))

    res = bass_utils.run_bass_kernel_spmd(nc, in_maps,
                                          core_ids=list(range(N_CORES)))
    logitsT = res.results[0]["logits_out"]          # [64, 256]
    return np.ascontiguousarray(logitsT.T)          # [256, 64]
